# revision 1
# baseline (speedup 1.0000x reference)
"""Trainium2 Bass kernel for TernaryLinear: y[b,m,n] = sum_k x[b,m,k] * w[k,n].

Shapes: x (4, 2048, 4096) fp32, w (4096, 4096) ternary fp32 -> y (4, 2048, 4096).

Strategy: flatten x to 8192 rows, row-shard across 8 NeuronCores (1024 rows
each), replicate w. Per core: keep x^T resident in SBUF as 32 k-tiles of
[128k x 1024m] fp16 (the stationary matmul operand; fp16 weights get the
fast-weight-load path so the per-matmul weight load hides under the previous
matmul), stream w as [128k x 512n] fp16 tiles (ternary {-1,0,1} is exact in
fp16), accumulate over K into 8 PSUM banks (one per 128-row m-tile) in fp32,
evict PSUM->SBUF alternating between the vector and scalar engines, DMA
results out. No cross-core communication; host concatenates the row shards.
"""

import sys

for _p in ("/opt/trn_rl_repo", "/opt/pypackages"):
    if _p not in sys.path:
        sys.path.append(_p)

import numpy as np

import concourse.bass as bass
import concourse.bacc as bacc
import concourse.mybir as mybir
import concourse.tile as tile
from concourse.bass_utils import run_bass_kernel_spmd

P = 128
NCORES = 8
B, M, K, N = 4, 2048, 4096, 4096
R = B * M            # 8192 rows total
MR = R // NCORES     # 1024 rows per core
KT = K // P          # 32 k-tiles
MT = MR // P         # 8 m-tiles per core
NCH = 512            # moving free dim per matmul (one PSUM bank of fp32)
NCHUNKS = N // NCH   # 8
F32 = mybir.dt.float32
F16 = mybir.dt.float16

_PROGRAM = None


def _build_program():
    nc = bacc.Bacc(
        "TRN2",
        target_bir_lowering=False,
        debug=False,
        num_devices=NCORES,
    )
    xt = nc.dram_tensor("xt", [P, KT, MT, P], F16, kind="ExternalInput").ap()
    w = nc.dram_tensor("w", [NCHUNKS, KT, P, NCH], F16, kind="ExternalInput").ap()
    y = nc.dram_tensor("y", [MT, P, N], F32, kind="ExternalOutput").ap()

    with tile.TileContext(nc) as tc:
        with (
            tc.tile_pool(name="xres", bufs=1) as xpool,
            tc.tile_pool(name="wstream", bufs=10) as wpool,
            tc.tile_pool(name="outstage", bufs=8) as opool,
            tc.tile_pool(name="acc", bufs=8, space="PSUM") as ppool,
        ):
            # x^T resident: one tile per k-tile, [128 kp, MT, 128 m]. The
            # loads are interleaved with the first n-chunk's w stream (same
            # DMA issue queue) so the PE starts after one x slice + one w
            # tile instead of after the whole x preload.
            xtiles = [None] * KT

            def evict(nch, mt, ps):
                ot = opool.tile([P, NCH], F32, tag="o", name=f"o{nch}_{mt}")
                if mt % 2 == 0:
                    nc.vector.tensor_copy(ot[:], ps[:])
                else:
                    nc.scalar.copy(ot[:], ps[:])
                # Alternate output DMAs across two HWDGE queues so they don't
                # serialize behind each other (or the w-input stream).
                dma_eng = nc.scalar if mt % 2 == 0 else nc.sync
                dma_eng.dma_start(out=y[mt, :, bass.ts(nch, NCH)], in_=ot[:])

            for nch in range(NCHUNKS - 1):
                psums = [
                    ppool.tile([P, NCH], F32, tag="acc", name=f"ps{nch}_{mt}")
                    for mt in range(MT)
                ]
                for kt in range(KT):
                    if nch == 0:
                        xtile = xpool.tile(
                            [P, MT, P], F16, tag=f"x{kt}", name=f"x{kt}"
                        )
                        nc.sync.dma_start(out=xtile[:], in_=xt[:, kt])
                        xtiles[kt] = xtile
                    wt = wpool.tile([P, NCH], F16, tag="w", name=f"w{nch}_{kt}")
                    # During n-chunk 0 the sync queue is busy with the x
                    # preload; issue w loads on the scalar queue in parallel.
                    (nc.scalar if nch == 0 else nc.sync).dma_start(
                        out=wt[:], in_=w[nch, kt]
                    )
                    for mt in range(MT):
                        nc.tensor.matmul(
                            out=psums[mt][:],
                            lhsT=xtiles[kt][:, mt, :],
                            rhs=wt[:],
                            start=(kt == 0),
                            stop=(kt == KT - 1),
                        )
                for mt in range(MT):
                    evict(nch, mt, psums[mt])

            # Last n-chunk: mt-outer / kt-inner so each m-tile's accumulation
            # finishes early and its eviction + output DMA overlap the
            # remaining matmul stream; only the last m-tile drains after the
            # final matmul. Needs all 32 w tiles live at once (own slots).
            nch = NCHUNKS - 1
            wlast = []
            for kt in range(KT):
                wt = wpool.tile(
                    [P, NCH], F16, tag=f"wl{kt}", name=f"wl{kt}", bufs=1
                )
                nc.sync.dma_start(out=wt[:], in_=w[nch, kt])
                wlast.append(wt)
            for mt in range(MT):
                ps = ppool.tile([P, NCH], F32, tag="acc", name=f"psL_{mt}")
                for kt in range(KT):
                    nc.tensor.matmul(
                        out=ps[:],
                        lhsT=xtiles[kt][:, mt, :],
                        rhs=wlast[kt][:],
                        start=(kt == 0),
                        stop=(kt == KT - 1),
                    )
                evict(nch, mt, ps)
    nc.compile()
    return nc


def _get_program():
    global _PROGRAM
    if _PROGRAM is None:
        _PROGRAM = _build_program()
    return _PROGRAM


def _prepare_in_maps(x: np.ndarray, w: np.ndarray):
    x = np.ascontiguousarray(x, dtype=np.float32)
    w = np.ascontiguousarray(w, dtype=np.float32)
    # x rows -> [core, mt, mp, kt, kp] -> [core, kp, kt, mt, mp], fp16
    xr = x.reshape(NCORES, MT, P, KT, P)
    xt_all = np.ascontiguousarray(
        xr.transpose(0, 4, 3, 1, 2).astype(np.float16)
    )
    # w [kt, kp, nch, nn] -> [nch, kt, kp, nn], fp16 (exact for ternary)
    wr = np.ascontiguousarray(
        w.reshape(KT, P, NCHUNKS, NCH).transpose(2, 0, 1, 3).astype(np.float16)
    )
    return [{"xt": xt_all[c], "w": wr} for c in range(NCORES)]


def _gather_output(results):
    y = np.stack([np.asarray(r["y"]) for r in results])  # [core, MT, P, N]
    return y.reshape(B, M, N)


def run(x: np.ndarray, w: np.ndarray, trace: bool = False):
    """Returns (y, BassKernelResults)."""
    nc = _get_program()
    in_maps = _prepare_in_maps(x, w)
    res = run_bass_kernel_spmd(
        nc, in_maps, core_ids=list(range(NCORES)), trace=trace
    )
    return _gather_output(res.results), res


def kernel(x: np.ndarray, w: np.ndarray) -> np.ndarray:
    y, _ = run(x, w, trace=False)
    return y



# revision 2
# speedup vs baseline: 1.2373x; 1.2373x over previous
"""Trainium2 Bass kernel for TernaryLinear: y[b,m,n] = sum_k x[b,m,k] * w[k,n].

Shapes: x (4, 2048, 4096) fp32, w (4096, 4096) ternary fp32 -> y (4, 2048, 4096).

Strategy: flatten x to 8192 rows, row-shard across 8 NeuronCores (1024 rows
each), replicate w. All matmuls run in fp8e4m3 with DoubleRow perf mode
(0.5 cycles/row, 256-deep contraction per matmul), which the ternary weight
permits exactly. Precision: x is decomposed on host as x ~= x_hi + x_lo/32
with x_hi = e4m3(x) and x_lo = e4m3(32*(x - x_hi)); the hi pass contracts all
32 k-tiles as 16 DoubleRow pairs against w, and a correction pass contracts
the first NLO k-tiles as NLO/2 pairs of x_lo against w/32 (ternary*2^-5 is
exact in e4m3, and the 2^5 scale on x_lo keeps the residual in fp8 normal
range). PSUM accumulates everything in fp32; results are evicted as bf16 and
upcast on host. No cross-core communication; host concatenates row shards.
"""

import sys

for _p in ("/opt/trn_rl_repo", "/opt/pypackages"):
    if _p not in sys.path:
        sys.path.append(_p)

import ml_dtypes
import numpy as np

import concourse.bass as bass
import concourse.bacc as bacc
import concourse.mybir as mybir
import concourse.tile as tile
from concourse.bass_utils import run_bass_kernel_spmd

P = 128
NCORES = 8
B, M, K, N = 4, 2048, 4096, 4096
R = B * M            # 8192 rows total
MR = R // NCORES     # 1024 rows per core
KT = K // P          # 32 k-tiles
MT = MR // P         # 8 m-tiles per core
NCH = 512            # moving free dim per matmul (one PSUM bank of fp32)
NCHUNKS = N // NCH   # 8
KP = KT // 2         # 16 DoubleRow k-tile pairs for the hi pass
NLO = 20             # k-tiles receiving the lo correction (rel err ~1.6e-2)
LP = NLO // 2        # 10 DoubleRow pairs for the lo pass
LO_SCALE = 32.0      # x_lo premultiplier; 1/32 folded into the w copy
F32 = mybir.dt.float32
BF16 = mybir.dt.bfloat16
F8 = mybir.dt.float8e4
E4M3 = ml_dtypes.float8_e4m3
DR = mybir.MatmulPerfMode.DoubleRow

_PROGRAM = None


def _build_program():
    nc = bacc.Bacc(
        "TRN2",
        target_bir_lowering=False,
        debug=False,
        num_devices=NCORES,
    )
    xhi = nc.dram_tensor("xhi", [P, KP, 2, MT, P], F8, kind="ExternalInput").ap()
    xlo = nc.dram_tensor("xlo", [P, LP, 2, MT, P], F8, kind="ExternalInput").ap()
    w2 = nc.dram_tensor("w2", [NCHUNKS, KP, P, 2, NCH], F8, kind="ExternalInput").ap()
    ws2 = nc.dram_tensor("ws2", [NCHUNKS, LP, P, 2, NCH], F8, kind="ExternalInput").ap()
    y = nc.dram_tensor("y", [MT, P, N], BF16, kind="ExternalOutput").ap()

    with tile.TileContext(nc) as tc:
        with (
            tc.tile_pool(name="xres", bufs=1) as xpool,
            tc.tile_pool(name="wstream", bufs=16) as wpool,
            tc.tile_pool(name="outstage", bufs=8) as opool,
            tc.tile_pool(name="acc", bufs=8, space="PSUM") as ppool,
        ):
            # x hi/lo resident: one tile per DoubleRow pair, [128 kp, 2, MT,
            # 128 m]. Loads are interleaved with the first n-chunk's w stream
            # (different DMA issue queues) so the PE starts after one x pair
            # + one w tile instead of after the whole x preload.
            xhi_t = [None] * KP
            xlo_t = [None] * LP

            def evict(nch, mt, ps):
                ot = opool.tile([P, NCH], BF16, tag="o", name=f"o{nch}_{mt}")
                if mt % 2 == 0:
                    nc.vector.tensor_copy(ot[:], ps[:])
                else:
                    nc.scalar.copy(ot[:], ps[:])
                # Alternate output DMAs across two HWDGE queues so they don't
                # serialize behind each other (or the w-input stream).
                dma_eng = nc.scalar if mt % 2 == 0 else nc.sync
                dma_eng.dma_start(out=y[mt, :, bass.ts(nch, NCH)], in_=ot[:])

            for nch in range(NCHUNKS):
                psums = [
                    ppool.tile([P, NCH], F32, tag="acc", name=f"ps{nch}_{mt}")
                    for mt in range(MT)
                ]
                for j in range(KP):
                    if nch == 0:
                        xt = xpool.tile(
                            [P, 2, MT, P], F8, tag=f"xh{j}", name=f"xh{j}"
                        )
                        nc.sync.dma_start(out=xt[:], in_=xhi[:, j])
                        xhi_t[j] = xt
                    wt = wpool.tile([P, 2, NCH], F8, tag="w", name=f"w{nch}_{j}")
                    # During n-chunk 0 the sync queue is busy with the x
                    # preload; issue w loads on the scalar queue in parallel.
                    (nc.scalar if nch == 0 else nc.sync).dma_start(
                        out=wt[:], in_=w2[nch, j]
                    )
                    for mt in range(MT):
                        nc.tensor.matmul(
                            out=psums[mt][:],
                            lhsT=xhi_t[j][:, :, mt, :],
                            rhs=wt[:],
                            start=(j == 0),
                            stop=False,
                            perf_mode=DR,
                        )
                for j in range(LP):
                    if nch == 0:
                        xt = xpool.tile(
                            [P, 2, MT, P], F8, tag=f"xl{j}", name=f"xl{j}"
                        )
                        nc.sync.dma_start(out=xt[:], in_=xlo[:, j])
                        xlo_t[j] = xt
                    wt = wpool.tile([P, 2, NCH], F8, tag="w", name=f"ws{nch}_{j}")
                    (nc.scalar if nch == 0 else nc.sync).dma_start(
                        out=wt[:], in_=ws2[nch, j]
                    )
                    for mt in range(MT):
                        nc.tensor.matmul(
                            out=psums[mt][:],
                            lhsT=xlo_t[j][:, :, mt, :],
                            rhs=wt[:],
                            start=False,
                            stop=(j == LP - 1),
                            perf_mode=DR,
                        )
                for mt in range(MT):
                    evict(nch, mt, psums[mt])
    nc.compile()
    return nc


def _get_program():
    global _PROGRAM
    if _PROGRAM is None:
        _PROGRAM = _build_program()
    return _PROGRAM


def _prepare_in_maps(x: np.ndarray, w: np.ndarray):
    x = np.ascontiguousarray(x, dtype=np.float32).reshape(R, K)
    w = np.ascontiguousarray(w, dtype=np.float32)

    x_hi8 = x.astype(E4M3)
    x_hi = x_hi8.astype(np.float32)
    x_lo8 = ((x[:, : NLO * P] - x_hi[:, : NLO * P]) * LO_SCALE).astype(E4M3)

    # x rows -> [core, kp, j, i, mt, mp], fp8
    def xt_layout(a, nkt):
        a = a.reshape(NCORES, MT, P, nkt, P).transpose(0, 4, 3, 1, 2)
        return np.ascontiguousarray(
            a.reshape(NCORES, P, nkt // 2, 2, MT, P)
        )

    xhi_all = xt_layout(x_hi8, KT)
    xlo_all = xt_layout(x_lo8, NLO)

    # w [kt(j,i), kp, nch, nn] -> [nch, j, kp, i, nn], fp8 (exact for ternary)
    def w_layout(a, npair):
        return np.ascontiguousarray(
            a.reshape(npair, 2, P, NCHUNKS, NCH).transpose(3, 0, 2, 1, 4)
        )

    w2_all = w_layout(w.astype(E4M3), KP)
    ws2_all = w_layout(
        (w[: NLO * P] * (1.0 / LO_SCALE)).astype(E4M3), LP
    )
    return [
        {"xhi": xhi_all[c], "xlo": xlo_all[c], "w2": w2_all, "ws2": ws2_all}
        for c in range(NCORES)
    ]


def _gather_output(results):
    y = np.stack([np.asarray(r["y"]) for r in results])  # [core, MT, P, N]
    return y.astype(np.float32).reshape(B, M, N)


def run(x: np.ndarray, w: np.ndarray, trace: bool = False):
    """Returns (y, BassKernelResults)."""
    nc = _get_program()
    in_maps = _prepare_in_maps(x, w)
    res = run_bass_kernel_spmd(
        nc, in_maps, core_ids=list(range(NCORES)), trace=trace
    )
    return _gather_output(res.results), res


def kernel(x: np.ndarray, w: np.ndarray) -> np.ndarray:
    y, _ = run(x, w, trace=False)
    return y


# revision 4
# speedup vs baseline: 1.3402x; 1.0832x over previous
"""Trainium2 Bass kernel for TernaryLinear: y[b,m,n] = sum_k x[b,m,k] * w[k,n].

Shapes: x (4, 2048, 4096) fp32, w (4096, 4096) ternary fp32 -> y (4, 2048, 4096).

Strategy: flatten x to 8192 rows, row-shard across 8 NeuronCores (1024 rows
each), replicate w. All matmuls run in fp8e4m3 with DoubleRow perf mode
(0.5 cycles/row, 256-deep contraction per matmul), which the ternary weight
permits exactly. Precision: x is decomposed on host as x ~= x_hi + x_lo/32
with x_hi = e4m3(x) and x_lo = e4m3(32*(x - x_hi)); the hi pass contracts all
32 k-tiles as 16 DoubleRow pairs against w, and a correction pass contracts
the first NLO k-tiles as NLO/2 pairs of x_lo against w/32 (ternary*2^-5 is
exact in e4m3, and the 2^5 scale on x_lo keeps the residual in fp8 normal
range). PSUM accumulates everything in fp32; results are evicted as bf16 and
upcast on host. No cross-core communication; host concatenates row shards.
"""

import sys

for _p in ("/opt/trn_rl_repo", "/opt/pypackages"):
    if _p not in sys.path:
        sys.path.append(_p)

import ml_dtypes
import numpy as np

import concourse.bass as bass
import concourse.bacc as bacc
import concourse.mybir as mybir
import concourse.tile as tile
from concourse.bass_utils import run_bass_kernel_spmd

P = 128
NCORES = 8
B, M, K, N = 4, 2048, 4096, 4096
R = B * M            # 8192 rows total
MR = R // NCORES     # 1024 rows per core
KT = K // P          # 32 k-tiles
MT = MR // P         # 8 m-tiles per core
NCH = 512            # moving free dim per matmul (one PSUM bank of fp32)
NCHUNKS = N // NCH   # 8
KP = KT // 2         # 16 DoubleRow k-tile pairs for the hi pass
NLO = 16             # k-tiles receiving the lo correction (rel err ~1.9e-2)
LP = NLO // 2        # 10 DoubleRow pairs for the lo pass
LO_SCALE = 32.0      # x_lo premultiplier; 1/32 folded into the w copy
F32 = mybir.dt.float32
BF16 = mybir.dt.bfloat16
F8 = mybir.dt.float8e4
E4M3 = ml_dtypes.float8_e4m3
DR = mybir.MatmulPerfMode.DoubleRow

_PROGRAM = None


def _build_program():
    nc = bacc.Bacc(
        "TRN2",
        target_bir_lowering=False,
        debug=False,
        num_devices=NCORES,
    )
    xhi = nc.dram_tensor("xhi", [P, KP, 2, MT, P], F8, kind="ExternalInput").ap()
    xlo = nc.dram_tensor("xlo", [P, LP, 2, MT, P], F8, kind="ExternalInput").ap()
    w2 = nc.dram_tensor("w2", [NCHUNKS, KP, P, 2, NCH], F8, kind="ExternalInput").ap()
    ws2 = nc.dram_tensor("ws2", [NCHUNKS, LP, P, 2, NCH], F8, kind="ExternalInput").ap()
    y = nc.dram_tensor("y", [MT, P, N], BF16, kind="ExternalOutput").ap()

    with tile.TileContext(nc) as tc:
        with (
            tc.tile_pool(name="xres", bufs=1) as xpool,
            tc.tile_pool(name="wstream", bufs=16) as wpool,
            tc.tile_pool(name="outstage", bufs=8) as opool,
            tc.tile_pool(name="acc", bufs=8, space="PSUM") as ppool,
        ):
            # x hi/lo resident: one tile per DoubleRow pair, [128 kp, 2, MT,
            # 128 m]. Loads are interleaved with the first n-chunk's w stream
            # (different DMA issue queues) so the PE starts after one x pair
            # + one w tile instead of after the whole x preload.
            xhi_t = [None] * KP
            xlo_t = [None] * LP

            def evict(nch, mt, ps):
                ot = opool.tile([P, NCH], BF16, tag="o", name=f"o{nch}_{mt}")
                if mt % 2 == 0:
                    nc.vector.tensor_copy(ot[:], ps[:])
                else:
                    nc.scalar.copy(ot[:], ps[:])
                # Alternate output DMAs across two HWDGE queues so they don't
                # serialize behind each other (or the w-input stream).
                dma_eng = nc.scalar if mt % 2 == 0 else nc.sync
                dma_eng.dma_start(out=y[mt, :, bass.ts(nch, NCH)], in_=ot[:])

            for nch in range(NCHUNKS - 1):
                psums = [
                    ppool.tile([P, NCH], F32, tag="acc", name=f"ps{nch}_{mt}")
                    for mt in range(MT)
                ]
                for j in range(KP):
                    if nch == 0:
                        xt = xpool.tile(
                            [P, 2, MT, P], F8, tag=f"xh{j}", name=f"xh{j}"
                        )
                        nc.sync.dma_start(out=xt[:], in_=xhi[:, j])
                        xhi_t[j] = xt
                    wt = wpool.tile([P, 2, NCH], F8, tag="w", name=f"w{nch}_{j}")
                    # During n-chunk 0 the sync queue is busy with the x
                    # preload; issue w loads on the scalar queue in parallel.
                    (nc.scalar if nch == 0 else nc.sync).dma_start(
                        out=wt[:], in_=w2[nch, j]
                    )
                    for mt in range(MT):
                        nc.tensor.matmul(
                            out=psums[mt][:],
                            lhsT=xhi_t[j][:, :, mt, :],
                            rhs=wt[:],
                            start=(j == 0),
                            stop=False,
                            perf_mode=DR,
                        )
                for j in range(LP):
                    if nch == 0:
                        xt = xpool.tile(
                            [P, 2, MT, P], F8, tag=f"xl{j}", name=f"xl{j}"
                        )
                        nc.sync.dma_start(out=xt[:], in_=xlo[:, j])
                        xlo_t[j] = xt
                    wt = wpool.tile([P, 2, NCH], F8, tag="w", name=f"ws{nch}_{j}")
                    (nc.scalar if nch == 0 else nc.sync).dma_start(
                        out=wt[:], in_=ws2[nch, j]
                    )
                    for mt in range(MT):
                        nc.tensor.matmul(
                            out=psums[mt][:],
                            lhsT=xlo_t[j][:, :, mt, :],
                            rhs=wt[:],
                            start=False,
                            stop=(j == LP - 1),
                            perf_mode=DR,
                        )
                for mt in range(MT):
                    evict(nch, mt, psums[mt])

            # Last n-chunk: mt-outer / pass-inner so each m-tile's
            # accumulation finishes early and its eviction + output DMA
            # overlap the remaining matmul stream; only the last m-tile
            # drains after the final matmul. Needs all 24 w tiles live at
            # once (own slots).
            nch = NCHUNKS - 1
            wlast = []
            for j in range(KP):
                wt = wpool.tile(
                    [P, 2, NCH], F8, tag=f"wl{j}", name=f"wl{j}", bufs=1
                )
                nc.sync.dma_start(out=wt[:], in_=w2[nch, j])
                wlast.append((xhi_t[j], wt))
            for j in range(LP):
                wt = wpool.tile(
                    [P, 2, NCH], F8, tag=f"wsl{j}", name=f"wsl{j}", bufs=1
                )
                nc.sync.dma_start(out=wt[:], in_=ws2[nch, j])
                wlast.append((xlo_t[j], wt))
            for mt in range(MT):
                ps = ppool.tile([P, NCH], F32, tag="acc", name=f"psL_{mt}")
                for i, (xt, wt) in enumerate(wlast):
                    nc.tensor.matmul(
                        out=ps[:],
                        lhsT=xt[:, :, mt, :],
                        rhs=wt[:],
                        start=(i == 0),
                        stop=(i == len(wlast) - 1),
                        perf_mode=DR,
                    )
                evict(nch, mt, ps)
    nc.compile()
    return nc


def _get_program():
    global _PROGRAM
    if _PROGRAM is None:
        _PROGRAM = _build_program()
    return _PROGRAM


def _prepare_in_maps(x: np.ndarray, w: np.ndarray):
    x = np.ascontiguousarray(x, dtype=np.float32).reshape(R, K)
    w = np.ascontiguousarray(w, dtype=np.float32)

    x_hi8 = x.astype(E4M3)
    x_hi = x_hi8.astype(np.float32)
    x_lo8 = ((x[:, : NLO * P] - x_hi[:, : NLO * P]) * LO_SCALE).astype(E4M3)

    # x rows -> [core, kp, j, i, mt, mp], fp8
    def xt_layout(a, nkt):
        a = a.reshape(NCORES, MT, P, nkt, P).transpose(0, 4, 3, 1, 2)
        return np.ascontiguousarray(
            a.reshape(NCORES, P, nkt // 2, 2, MT, P)
        )

    xhi_all = xt_layout(x_hi8, KT)
    xlo_all = xt_layout(x_lo8, NLO)

    # w [kt(j,i), kp, nch, nn] -> [nch, j, kp, i, nn], fp8 (exact for ternary)
    def w_layout(a, npair):
        return np.ascontiguousarray(
            a.reshape(npair, 2, P, NCHUNKS, NCH).transpose(3, 0, 2, 1, 4)
        )

    w2_all = w_layout(w.astype(E4M3), KP)
    ws2_all = w_layout(
        (w[: NLO * P] * (1.0 / LO_SCALE)).astype(E4M3), LP
    )
    return [
        {"xhi": xhi_all[c], "xlo": xlo_all[c], "w2": w2_all, "ws2": ws2_all}
        for c in range(NCORES)
    ]


def _gather_output(results):
    y = np.stack([np.asarray(r["y"]) for r in results])  # [core, MT, P, N]
    return y.astype(np.float32).reshape(B, M, N)


def run(x: np.ndarray, w: np.ndarray, trace: bool = False):
    """Returns (y, BassKernelResults)."""
    nc = _get_program()
    in_maps = _prepare_in_maps(x, w)
    res = run_bass_kernel_spmd(
        nc, in_maps, core_ids=list(range(NCORES)), trace=trace
    )
    return _gather_output(res.results), res


def kernel(x: np.ndarray, w: np.ndarray) -> np.ndarray:
    y, _ = run(x, w, trace=False)
    return y
